# revision 1
# baseline (speedup 1.0000x reference)
"""Last-query sparse attention on 8 TRN2 NeuronCores.

Reference computation (per sample b):
    prev  = x[b, :-1, :]                 # [T-1, D]
    final = x[b, -1, :]                  # [D]
    s     = prev @ final                 # [T-1]
    w     = softmax(s)
    att   = w @ prev                     # [D]
    out   = concat(final, att)           # [2D]

Sharding: batch (B=64) split 8 ways -> 8 samples per core, no collectives.

Per-core layout: x[b] ([4096, 256] f32) lands in SBUF as [128, 32, 256]
fp16 via SWDGE cast DMAs (partition p holds rows t = p*32 + i), 2 chunks
of 16 blocks per sample (samples 0/7: 4 half-chunk DMAs to shrink
pipeline ramp/tail). All DMA triggers + the tiny final-row gather run
on gpsimd, first thing.

Pass 1 (scores) entirely on DVE (gpsimd tensor ops trip the power
governor): fp16 mul (2x mode) + three pairwise tree-add levels +
segmented f32 reduce of the remaining 32. Self-score (t=4095) masked
via a bias column built with a 4-byte DMA (engine APs cannot address
partition 127).

Softmax: DVE row-max -> PE transpose (iota-built identity) -> DVE
reduce_max -> PE ones-matmul broadcast -> ACT copy with scale=-1 ->
ACT exp into fp16 weights. No gpsimd reduces at all.

Pass 2: 32 PE matmuls per sample ([128,1] weights x [128,256] block)
accumulating att in a [1,512] PSUM row, plus one ones-matmul writing
Z = sum(weights) at free [256:288]. ACT stages [1,288] to partition-0
stage[0,b]; per-sample epilogue: DVE 32-wide Z reduce + reciprocal,
ACT normalize copy (scale=1/Z) -- off the critical tail except sample
7's. One batched output DMA at the end; final-row halves are copied
HBM->HBM directly.
"""

import sys

sys.path.insert(0, "/opt/trn_rl_repo")

from contextlib import ExitStack

import numpy as np

import concourse.tile as tile
import concourse.bass_isa as bass_isa
from concourse import bacc, mybir
from concourse.bass_utils import run_bass_kernel_spmd

N_CORES = 8
B = 64
T = 4096
D = 256
BPC = B // N_CORES  # samples per core
P = 128
NBLK = T // P  # 32 blocks; t = p*NBLK + i
CB = 16  # blocks per chunk (2 chunks per sample)
F32 = mybir.dt.float32
FP16 = mybir.dt.float16
AX = mybir.AxisListType
ALU = mybir.AluOpType

_NC_CACHE = None


def _build():
    AF = mybir.ActivationFunctionType
    nc = bacc.Bacc(
        trn_type="TRN2",
        target_bir_lowering=False,
        debug=False,
        num_devices=N_CORES,
    )
    x_ext = nc.declare_dram_parameter("x", [BPC, T, D], F32, isOutput=False)
    out_ext = nc.declare_dram_parameter("out", [BPC, 2 * D], F32, isOutput=True)
    xap = x_ext.ap()
    oap = out_ext.ap()

    with ExitStack() as ctx:
        tc = ctx.enter_context(tile.TileContext(nc))
        xbpool = ctx.enter_context(tc.tile_pool(name="xbp", bufs=5))
        fpool = ctx.enter_context(tc.tile_pool(name="fp", bufs=2))
        scrpool = ctx.enter_context(tc.tile_pool(name="scr", bufs=3))
        spool = ctx.enter_context(tc.tile_pool(name="sp", bufs=3))
        stat = ctx.enter_context(tc.tile_pool(name="stat", bufs=3))
        cpool = ctx.enter_context(tc.tile_pool(name="const", bufs=1))
        pspool = ctx.enter_context(tc.tile_pool(name="ps", bufs=4, space="PSUM"))
        psf_pool = ctx.enter_context(tc.tile_pool(name="psf", bufs=2, space="PSUM"))

        xr = [xap[b].rearrange("(p i) d -> p i d", p=P) for b in range(BPC)]

        # --- earliest DMAs (gpsimd SWDGE; the casts require it anyway) -----
        Xh = [xbpool.tile([P, NBLK, D], FP16, tag="xh", name="xh") for _ in range(BPC)]

        def trig(b, lo, hi):
            nc.gpsimd.dma_start(Xh[b][:, lo:hi, :], xr[b][:, lo:hi, :])

        # first compute chunk ahead of everything, then the final-row gather
        trig(0, 0, 4)
        F_all = cpool.tile([1, BPC, D], F32, tag="fall")
        nc.gpsimd.dma_start(F_all[0:1, :, :], xap[:, T - 1, :].unsqueeze(0))
        for lo, hi in ((4, 8), (8, 16), (16, 24), (24, 32)):
            trig(0, lo, hi)
        trig(1, 0, CB)
        trig(1, CB, NBLK)

        # final half of the output: straight HBM->HBM copy (sync engine)
        nc.sync.dma_start(oap[:, 0:D], xap[:, T - 1, :])

        # --- constants on DVE ----------------------------------------------
        ones16 = cpool.tile([P, 1], FP16, tag="ones16")
        nc.vector.memset(ones16[:], 1.0)
        onesf = cpool.tile([1, P], F32, tag="onesf")
        nc.vector.memset(onesf[:], 1.0)
        maskbias = cpool.tile([P, 1], F32, tag="mb")
        nc.vector.memset(maskbias[:], 0.0)
        neg30 = cpool.tile([1, 1], F32, tag="neg30")
        nc.vector.memset(neg30[:], -1.0e30)
        # DMA can write partition 127 where engine APs cannot
        nc.sync.dma_start(maskbias[P - 1 : P, 0:1], neg30[0:1, 0:1])

        # partition-0 staging: [0:256] unnormalized att, [256:288] Z partials
        stage = cpool.tile([1, BPC, 288], F32, tag="stage")
        att_n = cpool.tile([1, BPC, D], F32, tag="attn")

        # Fh broadcast: PE ones-matmul + ACT cast copy
        Fh = [None] * BPC

        def fh_bcast(b):
            ps = psf_pool.tile([P, D], F32, tag="psf", name="psf")
            nc.tensor.matmul(ps[:], lhsT=onesf[:], rhs=F_all[0:1, b, :], start=True, stop=True)
            Fh[b] = fpool.tile([P, D], FP16, tag="fh", name="fh")
            nc.scalar.activation(Fh[b][:], ps[:], AF.Copy)

        fh_bcast(0)
        fh_bcast(1)

        def mul_l1(b, lo, hi, prod, l1):
            w = hi - lo
            s = lo - (lo // CB) * CB
            nc.vector.tensor_mul(
                prod[:, s : s + w, :],
                Xh[b][:, lo:hi, :],
                Fh[b][:].unsqueeze(1).broadcast_to((P, w, D)),
            )
            nc.vector.tensor_add(
                l1[:, s : s + w, :],
                prod[:, s : s + w, 0 : D // 2],
                prod[:, s : s + w, D // 2 : D],
            )

        rz = stat.tile([1, BPC], F32, tag="rz", bufs=1)

        def epilogue(b):
            # Z(b) -> 1/Z(b) on DVE (tiny); ACT does the normalize copy
            zw = CB if b == BPC - 1 else NBLK
            zb = stat.tile([1, 1], F32, tag="zb", name="zb")
            nc.vector.reduce_sum(zb[:], stage[0:1, b, D : D + zw], axis=AX.X)
            nc.vector.reciprocal(rz[0:1, b : b + 1], zb[:])
            nc.scalar.activation(
                att_n[0:1, b, :], stage[0:1, b, 0:D], AF.Copy,
                scale=rz[0:1, b : b + 1],
            )

        for b in range(BPC):
            halves = b in (0, BPC - 1)

            S = spool.tile([P, NBLK], F32, tag="s")
            prods = [scrpool.tile([P, CB, D], FP16, tag=f"prod{c}", name=f"prod{c}") for c in range(2)]
            l1s = [scrpool.tile([P, CB, D // 2], FP16, tag=f"l1{c}", name=f"l1{c}") for c in range(2)]
            l2s = [scrpool.tile([P, CB, D // 4], FP16, tag=f"l2{c}", name=f"l2{c}") for c in range(2)]
            l3s = [scrpool.tile([P, CB, D // 8], FP16, tag=f"l3{c}", name=f"l3{c}") for c in range(2)]

            def tree_red(c):
                nc.vector.tensor_add(
                    l2s[c][:], l1s[c][:, :, 0 : D // 4], l1s[c][:, :, D // 4 : D // 2]
                )
                nc.vector.tensor_add(
                    l3s[c][:], l2s[c][:, :, 0 : D // 8], l2s[c][:, :, D // 8 : D // 4]
                )
                nc.vector.reduce_sum(
                    S[:, c * CB : (c + 1) * CB], l3s[c][:], axis=AX.X
                )

            if b == 0:
                mul_l1(b, 0, 4, prods[0], l1s[0])
                mul_l1(b, 4, 8, prods[0], l1s[0])
                mul_l1(b, 8, 16, prods[0], l1s[0])
                tree_red(0)
                mul_l1(b, 16, 24, prods[1], l1s[1])
                mul_l1(b, 24, 32, prods[1], l1s[1])
                tree_red(1)
            elif halves:
                mul_l1(b, 0, 8, prods[0], l1s[0])
                mul_l1(b, 8, 16, prods[0], l1s[0])
                tree_red(0)
                # ---- flash tail, chunk A: local max softmax + matmuls ----
                rmA = stat.tile([P, 1], F32, tag="rmA", name="rmA")
                nc.vector.reduce_max(rmA[:], S[:, 0:CB], axis=AX.X)
                gmA = stat.tile([P, 1], F32, tag="gmA", name="gmA")
                nc.gpsimd.partition_all_reduce(
                    gmA[:], rmA[:], channels=P, reduce_op=bass_isa.ReduceOp.max
                )
                negA = stat.tile([P, 1], F32, tag="negA", name="negA")
                nc.scalar.activation(negA[:], gmA[:], AF.Copy, scale=-1.0)
                PwA = spool.tile([P, CB], FP16, tag="pwA", name="pwA")
                nc.scalar.activation(
                    PwA[:], S[:, 0:CB], AF.Exp, bias=negA[:], scale=1.0
                )
                bankA = pspool.tile([1, 512], F32, tag="attps", name="attpsA")
                for i in range(CB):
                    nc.tensor.matmul(
                        bankA[0:1, 0:D],
                        lhsT=PwA[:, i : i + 1],
                        rhs=Xh[b][:, i, :],
                        start=(i == 0),
                        stop=(i == CB - 1),
                    )
                nc.tensor.matmul(
                    bankA[0:1, D : D + CB], lhsT=ones16[:], rhs=PwA[:],
                    start=True, stop=True,
                )
                sbA = stat.tile([1, D + CB], F32, tag="sbA", name="sbA")
                nc.scalar.activation(sbA[:], bankA[0:1, 0 : D + CB], AF.Copy)
                mul_l1(b, 16, 24, prods[1], l1s[1])
                mul_l1(b, 24, 32, prods[1], l1s[1])
                tree_red(1)
            else:
                mul_l1(b, 0, CB, prods[0], l1s[0])
                tree_red(0)
                mul_l1(b, CB, NBLK, prods[1], l1s[1])
                tree_red(1)

            # prefetch triggers for sample b+2 (gpsimd queue)
            nxt = b + 2
            if 2 <= nxt < BPC:
                if nxt == BPC - 1:
                    for q in range(4):
                        trig(nxt, 8 * q, 8 * q + 8)
                else:
                    trig(nxt, 0, CB)
                    trig(nxt, CB, NBLK)

            if b == BPC - 1:
                # flash tail: chunk-A softmax/matmuls ran already (below);
                # here finish chunk B with the true max + rescale combine
                pass
            else:
                # mask self-score (t=4095 -> p=127, i=31), row max
                nc.vector.tensor_add(S[:, NBLK - 1 : NBLK], S[:, NBLK - 1 : NBLK], maskbias[:])
                rm = stat.tile([P, 1], F32, tag="rm")
                nc.vector.reduce_max(rm[:], S[:], axis=AX.X)

            if b < BPC - 1:
                # cross-partition max on gpsimd (idle but for DMA triggers);
                # result lands in every partition, negate via ACT copy scale=-1
                gmax = stat.tile([P, 1], F32, tag="gm")
                nc.gpsimd.partition_all_reduce(
                    gmax[:], rm[:], channels=P, reduce_op=bass_isa.ReduceOp.max
                )
                negmax = stat.tile([P, 1], F32, tag="nm")
                nc.scalar.activation(negmax[:], gmax[:], AF.Copy, scale=-1.0)

                Pw = spool.tile([P, NBLK], FP16, tag="pw")
                nc.scalar.activation(Pw[:], S[:], AF.Exp, bias=negmax[:], scale=1.0)

                # prefetch next sample's Fh (PE + ACT ahead of its muls)
                if b + 2 < BPC:
                    fh_bcast(b + 2)

                # pass 2: att row + Z into a [1, 512] psum row at partition 0
                bank = pspool.tile([1, 512], F32, tag="attps", name="attps")
                for i in range(NBLK):
                    nc.tensor.matmul(
                        bank[0:1, 0:D],
                        lhsT=Pw[:, i : i + 1],
                        rhs=Xh[b][:, i, :],
                        start=(i == 0),
                        stop=(i == NBLK - 1),
                    )
                nc.tensor.matmul(
                    bank[0:1, D : D + NBLK], lhsT=ones16[:], rhs=Pw[:], start=True, stop=True
                )
                nc.scalar.activation(stage[0:1, b, 0:288], bank[0:1, 0:288], AF.Copy)
            else:
                # ---- flash tail, chunk B: true max, rescale chunk-A psum ----
                nc.vector.tensor_add(
                    S[:, NBLK - 1 : NBLK], S[:, NBLK - 1 : NBLK], maskbias[:]
                )
                rmB = stat.tile([P, 1], F32, tag="rmB", name="rmB")
                nc.vector.reduce_max(rmB[:], S[:, CB:NBLK], axis=AX.X)
                rmF = stat.tile([P, 1], F32, tag="rmF", name="rmF")
                nc.vector.tensor_max(rmF[:], rmB[:], gmA[:])
                gmF = stat.tile([P, 1], F32, tag="gmF", name="gmF")
                nc.gpsimd.partition_all_reduce(
                    gmF[:], rmF[:], channels=P, reduce_op=bass_isa.ReduceOp.max
                )
                negF = stat.tile([P, 1], F32, tag="negF", name="negF")
                nc.scalar.activation(negF[:], gmF[:], AF.Copy, scale=-1.0)
                PwB = spool.tile([P, CB], FP16, tag="pwB", name="pwB")
                nc.scalar.activation(
                    PwB[:], S[:, CB:NBLK], AF.Exp, bias=negF[:], scale=1.0
                )
                # alpha = exp(gmA - M) on partition 0
                alpha = stat.tile([1, 1], F32, tag="alpha", name="alpha")
                nc.scalar.activation(
                    alpha[:], gmA[0:1, 0:1], AF.Exp, bias=negF[0:1, 0:1], scale=1.0
                )
                bankB = pspool.tile([1, 512], F32, tag="attps", name="attpsB")
                for i in range(CB, NBLK):
                    nc.tensor.matmul(
                        bankB[0:1, 0:D],
                        lhsT=PwB[:, i - CB : i - CB + 1],
                        rhs=Xh[b][:, i, :],
                        start=(i == CB),
                        stop=(i == NBLK - 1),
                    )
                nc.tensor.matmul(
                    bankB[0:1, D : D + CB], lhsT=ones16[:], rhs=PwB[:],
                    start=True, stop=True,
                )
                # stage = bankA * alpha + bankB (psum reads on DVE)
                nc.vector.scalar_tensor_tensor(
                    out=stage[0:1, b, 0 : D + CB],
                    in0=sbA[:],
                    scalar=alpha[0:1, 0:1],
                    in1=bankB[0:1, 0 : D + CB],
                    op0=ALU.mult,
                    op1=ALU.add,
                )
            if b > 0:
                epilogue(b - 1)

        epilogue(BPC - 1)
        # rows 0..6 can fly as soon as their normalize copies land (mid-flash);
        # only sample 7's 1KB row remains on the tail
        nc.sync.dma_start(
            oap[0 : BPC - 1, D : 2 * D].unsqueeze(0), att_n[0:1, 0 : BPC - 1, :]
        )
        nc.sync.dma_start(
            oap[BPC - 1 : BPC, D : 2 * D].unsqueeze(0), att_n[0:1, BPC - 1, :].unsqueeze(1)
        )

    nc.compile()
    return nc


def _run(x, trace=False):
    global _NC_CACHE
    x = np.ascontiguousarray(np.asarray(x, dtype=np.float32))
    assert x.shape == (B, T, D), x.shape
    if _NC_CACHE is None:
        _NC_CACHE = _build()
    in_maps = [{"x": x[c * BPC : (c + 1) * BPC]} for c in range(N_CORES)]
    res = run_bass_kernel_spmd(
        _NC_CACHE, in_maps, core_ids=list(range(N_CORES)), trace=trace
    )
    out = np.concatenate([res.results[c]["out"] for c in range(N_CORES)], axis=0)
    return out.astype(np.float32), res


def kernel(x):
    out, _ = _run(x, trace=False)
    return out

